# revision 43
# baseline (speedup 1.0000x reference)
"""Trainium2 Bass kernel for multi-head self-attention (dense transformer block).

Problem: x[4, 2048, 1024], w_qkv[3072, 1024], w_out[1024, 1024], b_out[1024]
  qkv = x @ w_qkv.T, rearranged 'b t (d k h) -> k b h t d' (k=3, h=16)
  attn = softmax(q @ k.T * DIM**-0.5); out = (attn @ v) concat heads @ w_out.T + b_out

Sharding (8 cores): data-parallel over batch b (4) x tensor-parallel over
head-groups (2 groups of 8 heads).  Each core gets x[b] (pre-transposed +
bf16-cast on host), the w_qkv rows for its 8 heads, and the matching w_out
columns; it produces a partial [T, DIM] output which the host sums per batch
pair (the "all-reduce" of the row-sharded w_out matmul) and adds b_out.

Schedule: the ScalarE exp stream (~267us busy) and the PE matmul stream
(~280us busy) are jointly critical, so the kernel starts exp as early as
possible and keeps both engines fed:
  phase A: stream x in 256-column chunks; kT for all heads ASAP, with qT for
           the first query chunks and chunk-wise scores+exp for the first two
           query chunks' units interleaved (ScalarE starts ~15us in).
  phase B: per unit (128-query chunk x head-pair): S^T = kT.T @ qT on PE, exp
           on ScalarE (softmax scale folded in), then expST.T @ [v|1]
           accumulation giving the attention output and its softmax
           denominator in one matmul chain.  The V projection, AV debt,
           PE transposes and the w_out projection are spread across unit
           slots as PE filler behind the exp stream; AVs run at each slot
           head so e-tile ring slots free before the slot's new allocation.
"""

import math
from collections import deque
from contextlib import ExitStack
from dataclasses import dataclass

import numpy as np

import concourse.bass as bass
import concourse.mybir as mybir
import concourse.tile as tile
from concourse import bacc
from concourse.bass_utils import run_bass_kernel_spmd
from concourse.masks import make_identity

F32 = mybir.dt.float32
BF16 = mybir.dt.bfloat16
P = 128


@dataclass(frozen=True)
class Cfg:
    T: int = 2048      # sequence length
    DIM: int = 1024    # model dim (= qkv contraction dim)
    NH: int = 8        # heads per core
    DH: int = 64       # head dim
    SCALE: float = 1024.0 ** -0.5
    ICSZ: int = 128    # query chunk (one unit = one ic x one head pair)
    TCH: int = 256     # phase-A x streaming chunk
    OCC: int = 512     # output-column chunk for the final projection
    N_VSLOT: int = 8   # phase-B slots that carry V chunks
    AV_PACE: int = 2   # max AV units retired per slot
    EP_BUFS: int = 30  # e-tile ring slots (2 per unit in flight)

    @property
    def CB(self):      # contraction blocks of 128 over DIM
        return self.DIM // P

    @property
    def OD(self):      # per-core attention width = NH*DH
        return self.NH * self.DH

    @property
    def OB(self):      # o-blocks of 128 (= head pairs, 2 x 64)
        return self.OD // P

    @property
    def JB(self):      # key blocks of 128
        return self.T // P

    @property
    def NIC(self):
        return self.T // self.ICSZ

    @property
    def NTCH(self):
        return self.T // self.TCH

    @property
    def NOCC(self):
        return self.DIM // self.OCC


def _emit_kernel(tc, cfg, xA, xQ, wq, wk, wv, woT, out):
    """Emit the per-core attention kernel under an open TileContext."""
    nc = tc.nc
    c = cfg
    VW = c.DH + 1  # per-head V width incl. ones column

    ctx = ExitStack()
    with ctx:
        persist = ctx.enter_context(tc.tile_pool(name="persist", bufs=1))
        mmp = ctx.enter_context(tc.tile_pool(name="mmp", bufs=2, space="PSUM"))
        smp = ctx.enter_context(tc.tile_pool(name="smp", bufs=4, space="PSUM"))
        ep = ctx.enter_context(tc.tile_pool(name="ep", bufs=c.EP_BUFS))
        qTp = ctx.enter_context(tc.tile_pool(name="qTp", bufs=5))

        kT_sb = persist.tile([P, c.OB, c.T], BF16, name="kT_sb", tag="kT")
        v_sb = persist.tile([P, c.JB, c.NH, VW], BF16, name="v_sb", tag="v")
        woT_sb = persist.tile([P, c.OB, c.DIM], BF16, name="woT_sb", tag="woT")
        wv_sb = persist.tile([P, c.CB, c.OD], BF16, name="wv_sb", tag="wv")
        wq_sb = persist.tile([P, c.CB, c.OD], BF16, name="wq_sb", tag="wq")
        ident = persist.tile([P, P], BF16, name="ident", tag="ident")

        make_identity(nc, ident)
        nc.gpsimd.memset(v_sb[:, :, :, c.DH : c.DH + 1], 1.0)

        e_tiles = {}    # (ic, hp) -> (e_half0, e_half1)
        qT_tiles = {}   # ic -> [P, OB, ICSZ]

        def emit_qT(ic, ob, x_t, toff):
            """One o-block of qT for query chunk ic; x_t holds [P, CB, *]."""
            if ic not in qT_tiles:
                qT_tiles[ic] = qTp.tile(
                    [P, c.OB, c.ICSZ], BF16, name=f"qT_{ic}", tag="qT"
                )
            ps = smp.tile([P, c.ICSZ], F32, name="ps_q", tag="sm")
            for cb in range(c.CB):
                nc.tensor.matmul(
                    ps,
                    wq_sb[:, cb, bass.ts(ob, P)],
                    x_t[:, cb, toff : toff + c.ICSZ],
                    start=(cb == 0),
                    stop=(cb == c.CB - 1),
                )
            nc.vector.tensor_copy(out=qT_tiles[ic][:, ob, :], in_=ps)

        def e_alloc(ic, hp):
            if (ic, hp) not in e_tiles:
                e_tiles[(ic, hp)] = tuple(
                    ep.tile([P, c.JB, c.ICSZ], BF16,
                            name=f"e_{ic}_{hp}_{half}", tag="e")
                    for half in range(2)
                )
            return e_tiles[(ic, hp)]

        def emit_scores_chunk(ic, hp, jb0, njb):
            """S^T then exp for keys jb0..jb0+njb of unit (ic, hp).
            Per-half 3D psum tiles: 4D PSUM tiles fail on hardware."""
            e_pair = e_alloc(ic, hp)
            qT = qT_tiles[ic]
            for half in range(2):
                rows = slice(64 * half, 64 * half + 64)
                ps = mmp.tile([P, njb, c.ICSZ], F32, name="ps_sc", tag="mm")
                for jj in range(njb):
                    jb = jb0 + jj
                    nc.tensor.matmul(
                        ps[:, jj, :],
                        kT_sb[rows, hp, bass.ts(jb, P)],
                        qT[rows, hp, :],
                        start=True,
                        stop=True,
                    )
                nc.scalar.activation(
                    out=e_pair[half][:, jb0 : jb0 + njb, :],
                    in_=ps,
                    func=mybir.ActivationFunctionType.Exp,
                    scale=c.SCALE,
                )

        def emit_scores_piece(ic, hp, g, half):
            """One 8-jb half-group of scores+exp for unit (ic, hp)."""
            e_pair = e_alloc(ic, hp)
            qT = qT_tiles[ic]
            rows = slice(64 * half, 64 * half + 64)
            ps = mmp.tile([P, 8, c.ICSZ], F32, name="ps_su", tag="mm")
            for jj in range(8):
                jb = 8 * g + jj
                nc.tensor.matmul(
                    ps[:, jj, :],
                    kT_sb[rows, hp, bass.ts(jb, P)],
                    qT[rows, hp, :],
                    start=True,
                    stop=True,
                )
            nc.scalar.activation(
                out=e_pair[half][:, 8 * g : 8 * g + 8, :],
                in_=ps,
                func=mybir.ActivationFunctionType.Exp,
                scale=c.SCALE,
            )

        def emit_scores_group(ic, hp, g):
            for half in range(2):
                emit_scores_piece(ic, hp, g, half)

        def emit_scores_unit(ic, hp):
            for g in range(2):
                emit_scores_group(ic, hp, g)

        # ---------------- phase A: kT + early scores ----------------
        with (
            tc.tile_pool(name="wkp", bufs=1) as wkp,
            tc.tile_pool(name="x0p", bufs=1) as x0p,
            tc.tile_pool(name="xp", bufs=3) as xp,
        ):
            wk_sb = wkp.tile([P, c.CB, c.OD], BF16, name="wk_sb", tag="wk")
            nc.sync.dma_start(out=wk_sb, in_=wk)

            x_tiles = []
            for tch in range(c.NTCH):
                pool = x0p if tch == 0 else xp
                x_t = pool.tile([P, c.CB, c.TCH], BF16, name=f"xA{tch}", tag="x")
                x_tiles.append(x_t)

            nc.sync.dma_start(out=x_tiles[0], in_=xA[0])
            nc.sync.dma_start(out=wq_sb, in_=wq)
            nc.sync.dma_start(out=x_tiles[1], in_=xA[1])
            nc.sync.dma_start(out=x_tiles[2], in_=xA[2])
            nc.sync.dma_start(out=wv_sb, in_=wv)
            nc.sync.dma_start(out=x_tiles[3], in_=xA[3])
            nc.sync.dma_start(out=woT_sb, in_=woT)

            for tch in range(c.NTCH):
                tsl = bass.ts(tch, c.TCH)
                x_t = x_tiles[tch]
                for ob in range(c.OB):
                    ps = smp.tile([P, c.TCH], F32, name="ps_k", tag="sm")
                    for cb in range(c.CB):
                        nc.tensor.matmul(
                            ps,
                            wk_sb[:, cb, bass.ts(ob, P)],
                            x_t[:, cb, :],
                            start=(cb == 0),
                            stop=(cb == c.CB - 1),
                        )
                    nc.vector.tensor_copy(out=kT_sb[:, ob, tsl], in_=ps)
                if tch == 0:
                    # qT for query chunk 0, then the earliest scores chunk
                    # (keys 0..255) so ScalarE starts ~13us in
                    for ob in range(c.OB):
                        emit_qT(0, ob, x_tiles[0], 0)
                    for hp in range(c.OB):
                        emit_scores_chunk(0, hp, 0, 2)
                if tch == 1:
                    for hp in range(c.OB):
                        emit_scores_chunk(0, hp, 2, 2)
                    for ob in range(c.OB):
                        emit_qT(1, ob, x_tiles[0], c.ICSZ)
                if tch == 2:
                    for hp in range(c.OB):
                        emit_scores_chunk(0, hp, 4, 2)
                    # qT for query chunks 2/3 from x chunk 1 (still live)
                    for ob in range(c.OB):
                        emit_qT(2, ob, x_tiles[1], 0)
                    for ob in range(c.OB):
                        emit_qT(3, ob, x_tiles[1], c.ICSZ)
                if tch == 3:
                    for hp in range(c.OB):
                        emit_scores_chunk(0, hp, 6, 2)
                    for hp in range(c.OB):
                        emit_scores_group(1, hp, 0)
                if tch == 5:
                    for hp in range(c.OB):
                        emit_scores_chunk(0, hp, 8, 4)
                if tch == 7:
                    for hp in range(c.OB):
                        emit_scores_chunk(0, hp, 12, 4)
                    for hp in range(c.OB):
                        emit_scores_group(1, hp, 1)
                # stream the next x chunk into the ring slot whose last
                # reader (kT, or the qT emissions above) is now emitted
                if tch >= 2 and tch + 2 < c.NTCH:
                    nc.sync.dma_start(
                        out=x_tiles[tch + 2], in_=xA[tch + 2]
                    )

        # ---------------- phase B: steady pipeline ----------------
        with (
            tc.tile_pool(name="xvp", bufs=3) as xvp,
            tc.tile_pool(name="xqp", bufs=2) as xqp,
            tc.tile_pool(name="ap", bufs=2) as ap,
            tc.tile_pool(name="atp", bufs=4) as atp,
            tc.tile_pool(name="op", bufs=2) as op,
            tc.tile_pool(name="rp", bufs=2) as rp,
        ):
            attn_tiles = {}
            attnT_tiles = {}
            pending_av = deque([(0, hp) for hp in range(c.OB)]
                               + [(1, hp) for hp in range(c.OB)])
            fin_queue = deque()
            xv_tiles = {}
            xq_cur = [None]

            def emit_xv_dma(vc):
                """DMA one 128-column x chunk for the V projection."""
                x_t = xvp.tile([P, c.CB, P], BF16, name=f"xV{vc}", tag="xv")
                tch, tbl = divmod(vc, 2)
                nc.sync.dma_start(
                    out=x_t, in_=xA[tch, :, :, tbl * P : (tbl + 1) * P]
                )
                xv_tiles[vc] = x_t

            def emit_v(vc):
                """One 128-row block of the V projection into v_sb."""
                x_t = xv_tiles.pop(vc)
                ps_v = smp.tile([P, c.OD], F32, name="ps_v", tag="sm")
                for cb in range(c.CB):
                    nc.tensor.matmul(
                        ps_v,
                        x_t[:, cb, :],
                        wv_sb[:, cb, :],
                        start=(cb == 0),
                        stop=(cb == c.CB - 1),
                    )
                nc.vector.tensor_copy(
                    out=v_sb[:, vc, :, 0 : c.DH],
                    in_=ps_v.rearrange("p (h d) -> p h d", h=c.NH),
                )

            def emit_av_half(ic, hp, half):
                """attn[i, dh] for head 2*hp+half of (ic, hp) + normalize."""
                if ic not in attn_tiles:
                    attn_tiles[ic] = ap.tile(
                        [P, c.OD], BF16, name=f"attn_{ic}", tag="attn"
                    )
                attn_sb = attn_tiles[ic]
                h = 2 * hp + half
                e = e_tiles[(ic, hp)][half]
                ps_av = smp.tile([P, VW], F32, name="ps_av", tag="sm")
                for jb in range(c.JB):
                    nc.tensor.matmul(
                        ps_av,
                        e[:, jb, :],
                        v_sb[:, jb, h, :],
                        start=(jb == 0),
                        stop=(jb == c.JB - 1),
                    )
                rec = rp.tile([P, 1], F32, name="rec", tag="rec")
                nc.vector.reciprocal(rec, ps_av[:, c.DH : c.DH + 1])
                nc.vector.tensor_scalar_mul(
                    out=attn_sb[:, bass.ts(h, c.DH)],
                    in0=ps_av[:, 0 : c.DH],
                    scalar1=rec,
                )

            def emit_av_tp(ic, hp):
                """PE-transpose the finished head pair's 128 columns."""
                e_tiles.pop((ic, hp))
                attn_sb = attn_tiles[ic]
                if ic not in attnT_tiles:
                    attnT_tiles[ic] = atp.tile(
                        [P, c.OB, c.ICSZ], BF16, name=f"attnT_{ic}", tag="attnT"
                    )
                ps_tp = smp.tile([P, P], BF16, name="ps_tp", tag="sm")
                nc.tensor.transpose(ps_tp, attn_sb[:, bass.ts(hp, P)], ident)
                nc.vector.tensor_copy(out=attnT_tiles[ic][:, hp, :], in_=ps_tp)
                if hp == c.OB - 1:
                    attn_tiles.pop(ic)
                    fin_queue.extend((ic, occ) for occ in range(c.NOCC))

            def emit_av(ic, hp):
                emit_av_half(ic, hp, 0)
                emit_av_half(ic, hp, 1)
                emit_av_tp(ic, hp)

            def av_thunks(ic, hp):
                def t0():
                    emit_av_half(ic, hp, 0)

                def t1():
                    emit_av_half(ic, hp, 1)
                    emit_av_tp(ic, hp)

                return [t0, t1]

            def emit_fin(ic, occ):
                attnT_sb = attnT_tiles[ic]
                ps_o = smp.tile([P, c.OCC], F32, name="ps_o", tag="sm")
                for ob in range(c.OB):
                    nc.tensor.matmul(
                        ps_o,
                        attnT_sb[:, ob, :],
                        woT_sb[:, ob, bass.ts(occ, c.OCC)],
                        start=(ob == 0),
                        stop=(ob == c.OB - 1),
                    )
                o_sb = op.tile([P, c.OCC], F32, name="o_sb", tag="ost")
                nc.vector.tensor_copy(out=o_sb, in_=ps_o)
                t0 = ic * c.ICSZ
                nc.sync.dma_start(
                    out=out[t0 : t0 + P, bass.ts(occ, c.OCC)], in_=o_sb
                )
                if occ == c.NOCC - 1:
                    attnT_tiles.pop(ic)

            emit_xv_dma(0)
            emit_xv_dma(1)
            emit_xv_dma(2)

            units = [(ic, hp) for ic in range(2, c.NIC) for hp in range(c.OB)]
            n_vc = 2 * c.NTCH  # 16 V chunks of 128 columns
            # Front-load V chunks: the early V-window slots run against the
            # phase-A exp backlog, so the PE surplus there is free
            v_sched = [3, 3, 2, 2, 2, 2, 2]
            v_next = [0]

            def emit_v_one():
                vc = v_next[0]
                v_next[0] += 1
                emit_v(vc)
                # ring: chunk vc+3 lands in the slot emit_v just freed
                if vc + 3 < n_vc:
                    emit_xv_dma(vc + 3)

            last_v = c.N_VSLOT - 1
            for idx, (ic, hp) in enumerate(units):
                # Collect this slot's PE filler as thunks, then interleave
                # them between the unit's score pieces: the PE stays busy
                # during the scores/exp lockstep (mmp ring depth 2) and the
                # exp stream never waits on a slot-sized PE burst.
                pre = []
                filler = []
                if idx >= last_v:
                    n_av = 0
                    while pending_av and n_av < c.AV_PACE:
                        u = pending_av.popleft()
                        th = av_thunks(*u)
                        # at the first AV slot, the unit whose e-ring slots
                        # this slot's allocation reuses must precede scores
                        if idx == last_v and n_av == 0:
                            pre.extend(th)
                        else:
                            filler.extend(th)
                        n_av += 1
                    if fin_queue and (len(pending_av) < 4 or idx % 3 == 0):
                        fq = fin_queue.popleft()
                        filler.append(lambda fq=fq: emit_fin(*fq))
                if hp == 0 and ic + 2 < c.NIC:
                    xq_cur[0] = xqp.tile(
                        [P, c.CB, c.ICSZ], BF16, name=f"xQ{ic + 2}", tag="xq"
                    )
                    nc.sync.dma_start(out=xq_cur[0], in_=xQ[ic + 2])
                nv = v_sched[idx] if idx < len(v_sched) else 0
                for _ in range(nv):
                    filler.append(emit_v_one)
                if ic + 2 < c.NIC:
                    filler.append(
                        lambda ic=ic, hp=hp: emit_qT(ic + 2, hp, xq_cur[0], 0)
                    )
                for th in pre:
                    th()
                fit = iter(filler)
                emit_scores_piece(ic, hp, 0, 0)
                emit_scores_piece(ic, hp, 0, 1)
                th = next(fit, None)
                if th is not None:
                    th()
                emit_scores_piece(ic, hp, 1, 0)
                th = next(fit, None)
                if th is not None:
                    th()
                emit_scores_piece(ic, hp, 1, 1)
                for th in fit:
                    th()
                pending_av.append((ic, hp))

            # tail: flush remaining AV debt and projections
            while pending_av:
                emit_av(*pending_av.popleft())
                if fin_queue:
                    emit_fin(*fin_queue.popleft())
            while fin_queue:
                emit_fin(*fin_queue.popleft())


def build_nc(cfg: Cfg = Cfg(), reps: int = 1):
    nc = bacc.Bacc()
    c = cfg
    xA = nc.declare_dram_parameter(
        "xA", [c.NTCH, P, c.CB, c.TCH], BF16, isOutput=False
    )
    xQ = nc.declare_dram_parameter(
        "xQ", [c.NIC, P, c.CB, c.ICSZ], BF16, isOutput=False
    )
    wq = nc.declare_dram_parameter("wq", [P, c.CB, c.OD], BF16, isOutput=False)
    wk = nc.declare_dram_parameter("wk", [P, c.CB, c.OD], BF16, isOutput=False)
    wv = nc.declare_dram_parameter("wv", [P, c.CB, c.OD], BF16, isOutput=False)
    woT = nc.declare_dram_parameter("woT", [P, c.OB, c.DIM], BF16, isOutput=False)
    out = nc.declare_dram_parameter("out", [c.T, c.DIM], F32, isOutput=True)
    with tile.TileContext(nc) as tc:
        for _ in range(reps):
            _emit_kernel(tc, cfg, xA[:], xQ[:], wq[:], wk[:], wv[:], woT[:], out[:])
    nc.finalize()
    return nc


def prepare_core_inputs(x, w_qkv, w_out, b, g, cfg: Cfg, n_groups: int):
    """Host-side shard prep for core (batch b, head-group g)."""
    import ml_dtypes

    c = cfg
    bf16 = ml_dtypes.bfloat16
    H = c.NH * n_groups
    d = np.arange(c.DH)
    heads = np.arange(c.NH * g, c.NH * (g + 1))

    # w_qkv row for (k, head h, dim d) is d*(3*H) + k*H + h
    def gather(k_idx):
        rows = (d[None, :] * (3 * H) + k_idx * H + heads[:, None]).reshape(-1)
        wT = w_qkv[rows, :].T.astype(bf16)  # [DIM, OD]
        return np.ascontiguousarray(
            wT.reshape(c.CB, P, c.OD).transpose(1, 0, 2)
        )

    xT = x[b].T.astype(bf16)  # [DIM, T]
    xA = np.ascontiguousarray(
        xT.reshape(c.CB, P, c.NTCH, c.TCH).transpose(2, 1, 0, 3)
    )
    xQ = np.ascontiguousarray(
        xT.reshape(c.CB, P, c.NIC, c.ICSZ).transpose(2, 1, 0, 3)
    )
    woTg = w_out[:, c.OD * g : c.OD * (g + 1)].T.astype(bf16)  # [OD, DIM]
    woT = np.ascontiguousarray(
        woTg.reshape(c.OB, P, c.DIM).transpose(1, 0, 2)
    )
    return {
        "xA": xA,
        "xQ": xQ,
        "wq": gather(0),
        "wk": gather(1),
        "wv": gather(2),
        "woT": woT,
    }


_NC_CACHE = {}


def _get_nc(cfg: Cfg):
    if cfg not in _NC_CACHE:
        _NC_CACHE[cfg] = build_nc(cfg)
    return _NC_CACHE[cfg]


def run(x, w_qkv, w_out, b_out, trace=False):
    """Shard, execute on 8 cores, gather. Returns (out, BassKernelResults)."""
    cfg = Cfg()
    B, T, DIM = x.shape
    assert (T, DIM) == (cfg.T, cfg.DIM), (x.shape, cfg)
    n_groups = 2
    nc = _get_nc(cfg)
    in_maps = [
        prepare_core_inputs(x, w_qkv, w_out, b, g, cfg, n_groups)
        for b in range(B)
        for g in range(n_groups)
    ]
    res = run_bass_kernel_spmd(
        nc, in_maps, core_ids=list(range(len(in_maps))), trace=trace
    )
    out = np.empty((B, T, DIM), dtype=np.float32)
    for b in range(B):
        out[b] = res.results[2 * b]["out"] + res.results[2 * b + 1]["out"]
    out += b_out.astype(np.float32)
    return out, res


def _make_pjrt_fn(nc, in_maps):
    """Build a non-donating jitted 8-core runner for a prebuilt nc."""
    import jax
    import numpy as np_
    from jax.sharding import Mesh, PartitionSpec
    from jax.experimental.shard_map import shard_map

    from concourse import bass2jax

    bass2jax.install_neuronx_cc_hook()
    n_cores = len(in_maps)
    partition_name = nc.partition_id_tensor.name if nc.partition_id_tensor else None
    in_names, out_names, out_avals, zero_outs = [], [], [], []
    for alloc in nc.m.functions[0].allocations:
        if not isinstance(alloc, mybir.MemoryLocationSet):
            continue
        name = alloc.memorylocations[0].name
        if alloc.kind == "ExternalInput":
            if name != partition_name:
                in_names.append(name)
        elif alloc.kind == "ExternalOutput":
            shape = tuple(alloc.tensor_shape)
            dtype = mybir.dt.np(alloc.dtype)
            out_names.append(name)
            out_avals.append(jax.core.ShapedArray(shape, dtype))
            zero_outs.append(np_.zeros(shape, dtype))
    n_params = len(in_names)
    all_in_names = in_names + out_names
    if partition_name is not None:
        all_in_names = all_in_names + [partition_name]

    def _body(*args):
        operands = list(args)
        if partition_name is not None:
            operands.append(bass2jax.partition_id_tensor())
        return tuple(
            bass2jax._bass_exec_p.bind(
                *operands,
                out_avals=tuple(out_avals),
                in_names=tuple(all_in_names),
                out_names=tuple(out_names),
                lowering_input_output_aliases=(),
                sim_require_finite=True,
                sim_require_nnan=True,
                nc=nc,
            )
        )

    devices = jax.devices()[:n_cores]
    mesh = Mesh(np_.asarray(devices), ("core",))
    nin = n_params + len(out_names)
    f = jax.jit(
        shard_map(
            _body,
            mesh=mesh,
            in_specs=(PartitionSpec("core"),) * nin,
            out_specs=(PartitionSpec("core"),) * len(out_names),
            check_rep=False,
        ),
        keep_unused=True,
    )
    concat_in = [
        np_.concatenate([np_.asarray(in_maps[c][n]) for c in range(n_cores)], axis=0)
        for n in in_names
    ] + [np_.zeros((n_cores * z.shape[0], *z.shape[1:]), z.dtype) for z in zero_outs]
    dev_in = jax.device_put(concat_in)
    return f, dev_in


def _time_fn(f, dev_in, calls=4, rounds=6):
    import time

    import jax

    r = f(*dev_in)
    jax.block_until_ready(r)
    best = float("inf")
    for _ in range(rounds):
        t0 = time.perf_counter()
        rs = [f(*dev_in) for _ in range(calls)]
        jax.block_until_ready(rs)
        best = min(best, (time.perf_counter() - t0) / calls)
    return best


def time_hw(x, w_qkv, w_out, b_out, reps=(4, 36)):
    """Marginal-cost HW timing: per-call time of an R2-repeat NEFF minus an
    R1-repeat NEFF, over (R2-R1), cancels the axon dispatch overhead."""
    cfg = Cfg()
    B = x.shape[0]
    in_maps = [
        prepare_core_inputs(x, w_qkv, w_out, b, g, cfg, 2)
        for b in range(B)
        for g in range(2)
    ]
    r1, r2 = reps
    ncA = build_nc(cfg, reps=r1)
    fA, devA = _make_pjrt_fn(ncA, in_maps)
    tA = _time_fn(fA, devA)
    ncB = build_nc(cfg, reps=r2)
    fB, devB = _make_pjrt_fn(ncB, in_maps)
    tB = _time_fn(fB, devB)
    per_exec = (tB - tA) / (r2 - r1)
    return tA, per_exec


def kernel(x, w_qkv, w_out, b_out):
    x = np.asarray(x, dtype=np.float32)
    w_qkv = np.asarray(w_qkv, dtype=np.float32)
    w_out = np.asarray(w_out, dtype=np.float32)
    b_out = np.asarray(b_out, dtype=np.float32)
    try:
        out, _ = run(x, w_qkv, w_out, b_out, trace=False)
    except Exception:
        # one retry for transient device errors
        out, _ = run(x, w_qkv, w_out, b_out, trace=False)
    return out


# revision 44
# speedup vs baseline: 1.3435x; 1.3435x over previous
"""Trainium2 Bass kernel for multi-head self-attention (dense transformer block).

Problem: x[4, 2048, 1024], w_qkv[3072, 1024], w_out[1024, 1024], b_out[1024]
  qkv = x @ w_qkv.T, rearranged 'b t (d k h) -> k b h t d' (k=3, h=16)
  attn = softmax(q @ k.T * DIM**-0.5); out = (attn @ v) concat heads @ w_out.T + b_out

Sharding (8 cores): data-parallel over batch b (4) x tensor-parallel over
head-groups (2 groups of 8 heads).  Each core gets x[b] (pre-transposed +
bf16-cast on host), the w_qkv rows for its 8 heads, and the matching w_out
columns; it produces a partial [T, DIM] output which the host sums per batch
pair (the "all-reduce" of the row-sharded w_out matmul) and adds b_out.

Schedule: the ScalarE exp stream (~267us busy) and the PE matmul stream
(~280us busy) are jointly critical, so the kernel starts exp as early as
possible and keeps both engines fed:
  phase A: stream x in 256-column chunks; kT for all heads ASAP, with qT for
           the first query chunks and chunk-wise scores+exp for the first two
           query chunks' units interleaved (ScalarE starts ~15us in).
  phase B: per unit (128-query chunk x head-pair): S^T = kT.T @ qT on PE, exp
           on ScalarE (softmax scale folded in), then expST.T @ [v|1]
           accumulation giving the attention output and its softmax
           denominator in one matmul chain.  The V projection, AV debt,
           PE transposes and the w_out projection are spread across unit
           slots as PE filler behind the exp stream; AVs run at each slot
           head so e-tile ring slots free before the slot's new allocation.
"""

import math
from collections import deque
from contextlib import ExitStack
from dataclasses import dataclass

import numpy as np

import concourse.bass as bass
import concourse.mybir as mybir
import concourse.tile as tile
from concourse import bacc
from concourse.bass_utils import run_bass_kernel_spmd
from concourse.masks import make_identity

F32 = mybir.dt.float32
BF16 = mybir.dt.bfloat16
P = 128


@dataclass(frozen=True)
class Cfg:
    T: int = 2048      # sequence length
    DIM: int = 1024    # model dim (= qkv contraction dim)
    NH: int = 8        # heads per core
    DH: int = 64       # head dim
    SCALE: float = 1024.0 ** -0.5
    ICSZ: int = 128    # query chunk (one unit = one ic x one head pair)
    TCH: int = 256     # phase-A x streaming chunk
    OCC: int = 512     # output-column chunk for the final projection
    N_VSLOT: int = 8   # phase-B slots that carry V chunks
    AV_PACE: int = 2   # max AV units retired per slot
    EP_BUFS: int = 30  # e-tile ring slots (2 per unit in flight)

    @property
    def CB(self):      # contraction blocks of 128 over DIM
        return self.DIM // P

    @property
    def OD(self):      # per-core attention width = NH*DH
        return self.NH * self.DH

    @property
    def OB(self):      # o-blocks of 128 (= head pairs, 2 x 64)
        return self.OD // P

    @property
    def JB(self):      # key blocks of 128
        return self.T // P

    @property
    def NIC(self):
        return self.T // self.ICSZ

    @property
    def NTCH(self):
        return self.T // self.TCH

    @property
    def NOCC(self):
        return self.DIM // self.OCC


def _emit_kernel(tc, cfg, xA, xQ, wq, wk, wv, woT, out):
    """Emit the per-core attention kernel under an open TileContext."""
    nc = tc.nc
    c = cfg
    VW = c.DH + 1  # per-head V width incl. ones column

    ctx = ExitStack()
    with ctx:
        persist = ctx.enter_context(tc.tile_pool(name="persist", bufs=1))
        mmp = ctx.enter_context(tc.tile_pool(name="mmp", bufs=2, space="PSUM"))
        smp = ctx.enter_context(tc.tile_pool(name="smp", bufs=4, space="PSUM"))
        ep = ctx.enter_context(tc.tile_pool(name="ep", bufs=c.EP_BUFS))
        qTp = ctx.enter_context(tc.tile_pool(name="qTp", bufs=5))

        kT_sb = persist.tile([P, c.OB, c.T], BF16, name="kT_sb", tag="kT")
        v_sb = persist.tile([P, c.JB, c.NH, VW], BF16, name="v_sb", tag="v")
        woT_sb = persist.tile([P, c.OB, c.DIM], BF16, name="woT_sb", tag="woT")
        wv_sb = persist.tile([P, c.CB, c.OD], BF16, name="wv_sb", tag="wv")
        wq_sb = persist.tile([P, c.CB, c.OD], BF16, name="wq_sb", tag="wq")
        ident = persist.tile([P, P], BF16, name="ident", tag="ident")

        make_identity(nc, ident)
        nc.gpsimd.memset(v_sb[:, :, :, c.DH : c.DH + 1], 1.0)

        e_tiles = {}    # (ic, hp) -> (e_half0, e_half1)
        qT_tiles = {}   # ic -> [P, OB, ICSZ]

        def emit_qT(ic, ob, x_t, toff):
            """One o-block of qT for query chunk ic; x_t holds [P, CB, *]."""
            if ic not in qT_tiles:
                qT_tiles[ic] = qTp.tile(
                    [P, c.OB, c.ICSZ], BF16, name=f"qT_{ic}", tag="qT"
                )
            ps = smp.tile([P, c.ICSZ], F32, name="ps_q", tag="sm")
            for cb in range(c.CB):
                nc.tensor.matmul(
                    ps,
                    wq_sb[:, cb, bass.ts(ob, P)],
                    x_t[:, cb, toff : toff + c.ICSZ],
                    start=(cb == 0),
                    stop=(cb == c.CB - 1),
                )
            nc.vector.tensor_copy(out=qT_tiles[ic][:, ob, :], in_=ps)

        def e_alloc(ic, hp):
            if (ic, hp) not in e_tiles:
                e_tiles[(ic, hp)] = tuple(
                    ep.tile([P, c.JB, c.ICSZ], BF16,
                            name=f"e_{ic}_{hp}_{half}", tag="e")
                    for half in range(2)
                )
            return e_tiles[(ic, hp)]

        def emit_scores_chunk(ic, hp, jb0, njb):
            """S^T then exp for keys jb0..jb0+njb of unit (ic, hp).
            Per-half 3D psum tiles: 4D PSUM tiles fail on hardware."""
            e_pair = e_alloc(ic, hp)
            qT = qT_tiles[ic]
            for half in range(2):
                rows = slice(64 * half, 64 * half + 64)
                ps = mmp.tile([P, njb, c.ICSZ], F32, name="ps_sc", tag="mm")
                for jj in range(njb):
                    jb = jb0 + jj
                    nc.tensor.matmul(
                        ps[:, jj, :],
                        kT_sb[rows, hp, bass.ts(jb, P)],
                        qT[rows, hp, :],
                        start=True,
                        stop=True,
                    )
                nc.scalar.activation(
                    out=e_pair[half][:, jb0 : jb0 + njb, :],
                    in_=ps,
                    func=mybir.ActivationFunctionType.Exp,
                    scale=c.SCALE,
                )

        def emit_scores_piece(ic, hp, g, half):
            """One 8-jb half-group of scores+exp for unit (ic, hp)."""
            e_pair = e_alloc(ic, hp)
            qT = qT_tiles[ic]
            rows = slice(64 * half, 64 * half + 64)
            ps = mmp.tile([P, 8, c.ICSZ], F32, name="ps_su", tag="mm")
            for jj in range(8):
                jb = 8 * g + jj
                nc.tensor.matmul(
                    ps[:, jj, :],
                    kT_sb[rows, hp, bass.ts(jb, P)],
                    qT[rows, hp, :],
                    start=True,
                    stop=True,
                )
            nc.scalar.activation(
                out=e_pair[half][:, 8 * g : 8 * g + 8, :],
                in_=ps,
                func=mybir.ActivationFunctionType.Exp,
                scale=c.SCALE,
            )

        def emit_scores_group(ic, hp, g):
            for half in range(2):
                emit_scores_piece(ic, hp, g, half)

        def emit_scores_unit(ic, hp):
            for g in range(2):
                emit_scores_group(ic, hp, g)

        # ---------------- phase A: kT + early scores ----------------
        with (
            tc.tile_pool(name="wkp", bufs=1) as wkp,
            tc.tile_pool(name="x0p", bufs=1) as x0p,
            tc.tile_pool(name="xp", bufs=3) as xp,
        ):
            wk_sb = wkp.tile([P, c.CB, c.OD], BF16, name="wk_sb", tag="wk")
            nc.sync.dma_start(out=wk_sb, in_=wk)

            x_tiles = []
            for tch in range(c.NTCH):
                pool = x0p if tch == 0 else xp
                x_t = pool.tile([P, c.CB, c.TCH], BF16, name=f"xA{tch}", tag="x")
                x_tiles.append(x_t)

            nc.sync.dma_start(out=x_tiles[0], in_=xA[0])
            nc.sync.dma_start(out=wq_sb, in_=wq)
            nc.sync.dma_start(out=x_tiles[1], in_=xA[1])
            nc.sync.dma_start(out=x_tiles[2], in_=xA[2])
            nc.sync.dma_start(out=wv_sb, in_=wv)
            nc.sync.dma_start(out=x_tiles[3], in_=xA[3])
            nc.sync.dma_start(out=woT_sb, in_=woT)

            for tch in range(c.NTCH):
                tsl = bass.ts(tch, c.TCH)
                x_t = x_tiles[tch]
                for ob in range(c.OB):
                    ps = smp.tile([P, c.TCH], F32, name="ps_k", tag="sm")
                    for cb in range(c.CB):
                        nc.tensor.matmul(
                            ps,
                            wk_sb[:, cb, bass.ts(ob, P)],
                            x_t[:, cb, :],
                            start=(cb == 0),
                            stop=(cb == c.CB - 1),
                        )
                    nc.vector.tensor_copy(out=kT_sb[:, ob, tsl], in_=ps)
                if tch == 0:
                    # qT for query chunk 0, then the earliest scores chunk
                    # (keys 0..255) so ScalarE starts ~13us in
                    for ob in range(c.OB):
                        emit_qT(0, ob, x_tiles[0], 0)
                    for hp in range(c.OB):
                        emit_scores_chunk(0, hp, 0, 2)
                if tch == 1:
                    for hp in range(c.OB):
                        emit_scores_chunk(0, hp, 2, 2)
                    for ob in range(c.OB):
                        emit_qT(1, ob, x_tiles[0], c.ICSZ)
                if tch == 2:
                    for hp in range(c.OB):
                        emit_scores_chunk(0, hp, 4, 2)
                    # qT for query chunks 2/3 from x chunk 1 (still live)
                    for ob in range(c.OB):
                        emit_qT(2, ob, x_tiles[1], 0)
                    for ob in range(c.OB):
                        emit_qT(3, ob, x_tiles[1], c.ICSZ)
                if tch == 3:
                    for hp in range(c.OB):
                        emit_scores_chunk(0, hp, 6, 2)
                    for hp in range(c.OB):
                        emit_scores_group(1, hp, 0)
                if tch == 5:
                    for hp in range(c.OB):
                        emit_scores_chunk(0, hp, 8, 4)
                if tch == 7:
                    for hp in range(c.OB):
                        emit_scores_chunk(0, hp, 12, 4)
                    for hp in range(c.OB):
                        emit_scores_group(1, hp, 1)
                # stream the next x chunk into the ring slot whose last
                # reader (kT, or the qT emissions above) is now emitted
                if tch >= 2 and tch + 2 < c.NTCH:
                    nc.sync.dma_start(
                        out=x_tiles[tch + 2], in_=xA[tch + 2]
                    )

        # ---------------- phase B: steady pipeline ----------------
        with (
            tc.tile_pool(name="xvp", bufs=3) as xvp,
            tc.tile_pool(name="xqp", bufs=2) as xqp,
            tc.tile_pool(name="ap", bufs=2) as ap,
            tc.tile_pool(name="atp", bufs=4) as atp,
            tc.tile_pool(name="op", bufs=2) as op,
            tc.tile_pool(name="rp", bufs=2) as rp,
        ):
            attn_tiles = {}
            attnT_tiles = {}
            pending_av = deque([(0, hp) for hp in range(c.OB)]
                               + [(1, hp) for hp in range(c.OB)])
            fin_queue = deque()
            xv_tiles = {}
            xq_cur = [None]

            def emit_xv_dma(vc):
                """DMA one 128-column x chunk for the V projection."""
                x_t = xvp.tile([P, c.CB, P], BF16, name=f"xV{vc}", tag="xv")
                tch, tbl = divmod(vc, 2)
                nc.sync.dma_start(
                    out=x_t, in_=xA[tch, :, :, tbl * P : (tbl + 1) * P]
                )
                xv_tiles[vc] = x_t

            def emit_v(vc):
                """One 128-row block of the V projection into v_sb."""
                x_t = xv_tiles.pop(vc)
                ps_v = smp.tile([P, c.OD], F32, name="ps_v", tag="sm")
                for cb in range(c.CB):
                    nc.tensor.matmul(
                        ps_v,
                        x_t[:, cb, :],
                        wv_sb[:, cb, :],
                        start=(cb == 0),
                        stop=(cb == c.CB - 1),
                    )
                nc.vector.tensor_copy(
                    out=v_sb[:, vc, :, 0 : c.DH],
                    in_=ps_v.rearrange("p (h d) -> p h d", h=c.NH),
                )

            def emit_av_half(ic, hp, half):
                """attn[i, dh] for head 2*hp+half of (ic, hp) + normalize."""
                if ic not in attn_tiles:
                    attn_tiles[ic] = ap.tile(
                        [P, c.OD], BF16, name=f"attn_{ic}", tag="attn"
                    )
                attn_sb = attn_tiles[ic]
                h = 2 * hp + half
                e = e_tiles[(ic, hp)][half]
                ps_av = smp.tile([P, VW], F32, name="ps_av", tag="sm")
                for jb in range(c.JB):
                    nc.tensor.matmul(
                        ps_av,
                        e[:, jb, :],
                        v_sb[:, jb, h, :],
                        start=(jb == 0),
                        stop=(jb == c.JB - 1),
                    )
                rec = rp.tile([P, 1], F32, name="rec", tag="rec")
                nc.vector.reciprocal(rec, ps_av[:, c.DH : c.DH + 1])
                nc.vector.tensor_scalar_mul(
                    out=attn_sb[:, bass.ts(h, c.DH)],
                    in0=ps_av[:, 0 : c.DH],
                    scalar1=rec,
                )

            def emit_av_tp(ic, hp):
                """PE-transpose the finished head pair's 128 columns."""
                e_tiles.pop((ic, hp))
                attn_sb = attn_tiles[ic]
                if ic not in attnT_tiles:
                    attnT_tiles[ic] = atp.tile(
                        [P, c.OB, c.ICSZ], BF16, name=f"attnT_{ic}", tag="attnT"
                    )
                ps_tp = smp.tile([P, P], BF16, name="ps_tp", tag="sm")
                nc.tensor.transpose(ps_tp, attn_sb[:, bass.ts(hp, P)], ident)
                nc.vector.tensor_copy(out=attnT_tiles[ic][:, hp, :], in_=ps_tp)
                if hp == c.OB - 1:
                    attn_tiles.pop(ic)
                    fin_queue.extend((ic, occ) for occ in range(c.NOCC))

            def emit_av(ic, hp):
                emit_av_half(ic, hp, 0)
                emit_av_half(ic, hp, 1)
                emit_av_tp(ic, hp)

            def av_thunks(ic, hp):
                def t0():
                    emit_av_half(ic, hp, 0)

                def t1():
                    emit_av_half(ic, hp, 1)
                    emit_av_tp(ic, hp)

                return [t0, t1]

            def emit_fin(ic, occ):
                attnT_sb = attnT_tiles[ic]
                ps_o = smp.tile([P, c.OCC], F32, name="ps_o", tag="sm")
                for ob in range(c.OB):
                    nc.tensor.matmul(
                        ps_o,
                        attnT_sb[:, ob, :],
                        woT_sb[:, ob, bass.ts(occ, c.OCC)],
                        start=(ob == 0),
                        stop=(ob == c.OB - 1),
                    )
                o_sb = op.tile([P, c.OCC], F32, name="o_sb", tag="ost")
                nc.vector.tensor_copy(out=o_sb, in_=ps_o)
                t0 = ic * c.ICSZ
                nc.sync.dma_start(
                    out=out[t0 : t0 + P, bass.ts(occ, c.OCC)], in_=o_sb
                )
                if occ == c.NOCC - 1:
                    attnT_tiles.pop(ic)

            emit_xv_dma(0)
            emit_xv_dma(1)
            emit_xv_dma(2)

            units = [(ic, hp) for ic in range(2, c.NIC) for hp in range(c.OB)]
            n_vc = 2 * c.NTCH  # 16 V chunks of 128 columns
            # Front-load V chunks: the early V-window slots run against the
            # phase-A exp backlog, so the PE surplus there is free
            v_sched = [3, 3, 2, 2, 2, 2, 2]
            v_next = [0]

            def emit_v_one():
                vc = v_next[0]
                v_next[0] += 1
                emit_v(vc)
                # ring: chunk vc+3 lands in the slot emit_v just freed
                if vc + 3 < n_vc:
                    emit_xv_dma(vc + 3)

            last_v = c.N_VSLOT - 1
            for idx, (ic, hp) in enumerate(units):
                # Collect this slot's PE filler as thunks, then interleave
                # them between the unit's score pieces: the PE stays busy
                # during the scores/exp lockstep (mmp ring depth 2) and the
                # exp stream never waits on a slot-sized PE burst.
                pre = []
                filler = []
                if idx >= last_v:
                    n_av = 0
                    while pending_av and n_av < c.AV_PACE:
                        u = pending_av.popleft()
                        th = av_thunks(*u)
                        # at the first AV slot, the unit whose e-ring slots
                        # this slot's allocation reuses must precede scores
                        if idx == last_v and n_av == 0:
                            pre.extend(th)
                        else:
                            filler.extend(th)
                        n_av += 1
                    if fin_queue and (len(pending_av) < 4 or idx % 3 == 0):
                        fq = fin_queue.popleft()
                        filler.append(lambda fq=fq: emit_fin(*fq))
                if hp == 0 and ic + 2 < c.NIC:
                    xq_cur[0] = xqp.tile(
                        [P, c.CB, c.ICSZ], BF16, name=f"xQ{ic + 2}", tag="xq"
                    )
                    nc.sync.dma_start(out=xq_cur[0], in_=xQ[ic + 2])
                nv = v_sched[idx] if idx < len(v_sched) else 0
                for _ in range(nv):
                    filler.append(emit_v_one)
                if ic + 2 < c.NIC:
                    filler.append(
                        lambda ic=ic, hp=hp: emit_qT(ic + 2, hp, xq_cur[0], 0)
                    )
                for th in pre:
                    th()
                fit = iter(filler)
                emit_scores_piece(ic, hp, 0, 0)
                emit_scores_piece(ic, hp, 0, 1)
                th = next(fit, None)
                if th is not None:
                    th()
                emit_scores_piece(ic, hp, 1, 0)
                th = next(fit, None)
                if th is not None:
                    th()
                emit_scores_piece(ic, hp, 1, 1)
                for th in fit:
                    th()
                pending_av.append((ic, hp))

            # tail: flush remaining AV debt and projections
            while pending_av:
                emit_av(*pending_av.popleft())
                if fin_queue:
                    emit_fin(*fin_queue.popleft())
            while fin_queue:
                emit_fin(*fin_queue.popleft())


def build_nc(cfg: Cfg = Cfg(), reps: int = 1):
    nc = bacc.Bacc()
    c = cfg
    xA = nc.declare_dram_parameter(
        "xA", [c.NTCH, P, c.CB, c.TCH], BF16, isOutput=False
    )
    xQ = nc.declare_dram_parameter(
        "xQ", [c.NIC, P, c.CB, c.ICSZ], BF16, isOutput=False
    )
    wq = nc.declare_dram_parameter("wq", [P, c.CB, c.OD], BF16, isOutput=False)
    wk = nc.declare_dram_parameter("wk", [P, c.CB, c.OD], BF16, isOutput=False)
    wv = nc.declare_dram_parameter("wv", [P, c.CB, c.OD], BF16, isOutput=False)
    woT = nc.declare_dram_parameter("woT", [P, c.OB, c.DIM], BF16, isOutput=False)
    out = nc.declare_dram_parameter("out", [c.T, c.DIM], F32, isOutput=True)
    with tile.TileContext(nc) as tc:
        for _ in range(reps):
            _emit_kernel(tc, cfg, xA[:], xQ[:], wq[:], wk[:], wv[:], woT[:], out[:])
    nc.finalize()
    return nc


def prepare_core_inputs(x, w_qkv, w_out, b, g, cfg: Cfg, n_groups: int):
    """Host-side shard prep for core (batch b, head-group g)."""
    import ml_dtypes

    c = cfg
    bf16 = ml_dtypes.bfloat16
    H = c.NH * n_groups
    d = np.arange(c.DH)
    heads = np.arange(c.NH * g, c.NH * (g + 1))

    # w_qkv row for (k, head h, dim d) is d*(3*H) + k*H + h
    def gather(k_idx):
        rows = (d[None, :] * (3 * H) + k_idx * H + heads[:, None]).reshape(-1)
        wT = w_qkv[rows, :].T.astype(bf16)  # [DIM, OD]
        return np.ascontiguousarray(
            wT.reshape(c.CB, P, c.OD).transpose(1, 0, 2)
        )

    xT = x[b].T.astype(bf16)  # [DIM, T]
    xA = np.ascontiguousarray(
        xT.reshape(c.CB, P, c.NTCH, c.TCH).transpose(2, 1, 0, 3)
    )
    xQ = np.ascontiguousarray(
        xT.reshape(c.CB, P, c.NIC, c.ICSZ).transpose(2, 1, 0, 3)
    )
    woTg = w_out[:, c.OD * g : c.OD * (g + 1)].T.astype(bf16)  # [OD, DIM]
    woT = np.ascontiguousarray(
        woTg.reshape(c.OB, P, c.DIM).transpose(1, 0, 2)
    )
    return {
        "xA": xA,
        "xQ": xQ,
        "wq": gather(0),
        "wk": gather(1),
        "wv": gather(2),
        "woT": woT,
    }


_NC_CACHE = {}


def _get_nc(cfg: Cfg):
    if cfg not in _NC_CACHE:
        _NC_CACHE[cfg] = build_nc(cfg)
    return _NC_CACHE[cfg]


def run(x, w_qkv, w_out, b_out, trace=False):
    """Shard, execute on 8 cores, gather. Returns (out, BassKernelResults)."""
    cfg = Cfg()
    B, T, DIM = x.shape
    assert (T, DIM) == (cfg.T, cfg.DIM), (x.shape, cfg)
    n_groups = 2
    nc = _get_nc(cfg)
    in_maps = [
        prepare_core_inputs(x, w_qkv, w_out, b, g, cfg, n_groups)
        for b in range(B)
        for g in range(n_groups)
    ]
    res = run_bass_kernel_spmd(
        nc, in_maps, core_ids=list(range(len(in_maps))), trace=trace
    )
    out = np.empty((B, T, DIM), dtype=np.float32)
    for b in range(B):
        out[b] = res.results[2 * b]["out"] + res.results[2 * b + 1]["out"]
    out += b_out.astype(np.float32)
    return out, res


def _make_pjrt_fn(nc, in_maps):
    """Build a non-donating jitted 8-core runner for a prebuilt nc."""
    import jax
    import numpy as np_
    from jax.sharding import Mesh, PartitionSpec
    from jax.experimental.shard_map import shard_map

    from concourse import bass2jax

    bass2jax.install_neuronx_cc_hook()
    n_cores = len(in_maps)
    partition_name = nc.partition_id_tensor.name if nc.partition_id_tensor else None
    in_names, out_names, out_avals, zero_outs = [], [], [], []
    for alloc in nc.m.functions[0].allocations:
        if not isinstance(alloc, mybir.MemoryLocationSet):
            continue
        name = alloc.memorylocations[0].name
        if alloc.kind == "ExternalInput":
            if name != partition_name:
                in_names.append(name)
        elif alloc.kind == "ExternalOutput":
            shape = tuple(alloc.tensor_shape)
            dtype = mybir.dt.np(alloc.dtype)
            out_names.append(name)
            out_avals.append(jax.core.ShapedArray(shape, dtype))
            zero_outs.append(np_.zeros(shape, dtype))
    n_params = len(in_names)
    all_in_names = in_names + out_names
    if partition_name is not None:
        all_in_names = all_in_names + [partition_name]

    def _body(*args):
        operands = list(args)
        if partition_name is not None:
            operands.append(bass2jax.partition_id_tensor())
        return tuple(
            bass2jax._bass_exec_p.bind(
                *operands,
                out_avals=tuple(out_avals),
                in_names=tuple(all_in_names),
                out_names=tuple(out_names),
                lowering_input_output_aliases=(),
                sim_require_finite=True,
                sim_require_nnan=True,
                nc=nc,
            )
        )

    devices = jax.devices()[:n_cores]
    mesh = Mesh(np_.asarray(devices), ("core",))
    nin = n_params + len(out_names)
    f = jax.jit(
        shard_map(
            _body,
            mesh=mesh,
            in_specs=(PartitionSpec("core"),) * nin,
            out_specs=(PartitionSpec("core"),) * len(out_names),
            check_rep=False,
        ),
        keep_unused=True,
    )
    concat_in = [
        np_.concatenate([np_.asarray(in_maps[c][n]) for c in range(n_cores)], axis=0)
        for n in in_names
    ] + [np_.zeros((n_cores * z.shape[0], *z.shape[1:]), z.dtype) for z in zero_outs]
    dev_in = jax.device_put(concat_in)
    return f, dev_in


def _time_fn(f, dev_in, calls=4, rounds=6):
    import time

    import jax

    r = f(*dev_in)
    jax.block_until_ready(r)
    best = float("inf")
    for _ in range(rounds):
        t0 = time.perf_counter()
        rs = [f(*dev_in) for _ in range(calls)]
        jax.block_until_ready(rs)
        best = min(best, (time.perf_counter() - t0) / calls)
    return best


def time_hw(x, w_qkv, w_out, b_out, reps=(4, 36), passes=3):
    """Marginal-cost HW timing: per-call time of an R2-repeat NEFF minus an
    R1-repeat NEFF, over (R2-R1), cancels the axon dispatch overhead.  The
    axon proxy's dispatch time drifts by tens of us between model loads, so
    the A/B pair is measured `passes` times and the median marginal is
    returned."""
    cfg = Cfg()
    B = x.shape[0]
    in_maps = [
        prepare_core_inputs(x, w_qkv, w_out, b, g, cfg, 2)
        for b in range(B)
        for g in range(2)
    ]
    r1, r2 = reps
    ncA = build_nc(cfg, reps=r1)
    fA, devA = _make_pjrt_fn(ncA, in_maps)
    ncB = build_nc(cfg, reps=r2)
    fB, devB = _make_pjrt_fn(ncB, in_maps)
    marginals = []
    tA = None
    for _ in range(passes):
        tA_i = _time_fn(fA, devA)
        tB_i = _time_fn(fB, devB)
        tA = tA_i if tA is None else min(tA, tA_i)
        marginals.append((tB_i - tA_i) / (r2 - r1))
    marginals.sort()
    per_exec = marginals[len(marginals) // 2]
    return tA, per_exec


def kernel(x, w_qkv, w_out, b_out):
    x = np.asarray(x, dtype=np.float32)
    w_qkv = np.asarray(w_qkv, dtype=np.float32)
    w_out = np.asarray(w_out, dtype=np.float32)
    b_out = np.asarray(b_out, dtype=np.float32)
    try:
        out, _ = run(x, w_qkv, w_out, b_out, trace=False)
    except Exception:
        # one retry for transient device errors
        out, _ = run(x, w_qkv, w_out, b_out, trace=False)
    return out
